# revision 5
# baseline (speedup 1.0000x reference)
"""Trainium2 Bass kernel for nn_NodeDecoder (sparse_attention).

Reference computation (B=256, V=16, N=1024, D=512):
    cat    = concat([g_node, Z_veh.mean(1), g_graph], -1)          # [B, 3D]
    ctx    = relu(cat @ W_ctx.T + b_ctx)                           # [B, D]
    Q      = ctx @ Wq.T                                            # [B, D]
    K      = Z_node @ Wk.T                                         # [B, N, D]
    logits = CLIP * tanh((Q . K) / sqrt(D)), masked to -inf        # [B, N]

Key algebraic transform: Q . (Z_node @ Wk.T) == (Q @ Wk) . Z_node, so the
B*N*D*D einsum collapses to a [B,D]@[D,D] matmul plus a B*N*D dot-product
sweep.  The kernel is then HBM-bandwidth-bound on streaming Z_node once.

Distribution: data-parallel over batch B across 8 NeuronCores (32 b/core),
small weights replicated.  All FLOPs (including the Z_veh mean, realized as
accumulated PE transposes) run on device; the host only slices/relayouts
inputs and reassembles the output (masked positions filled with -inf).

Per-core dataflow:
  - build cat.T via PE transposes (identity matmuls); Z_veh mean becomes 16
    accumulated transposes with the 1/16 folded into W_ctx rows host-side
  - chain ctx.T -> Q.T -> qtld.T = (Q @ Wk/sqrt(D)).T via PE matmuls with
    contraction on partitions (weights pre-transposed host-side: pure layout)
  - main loop over b: stream Z_node[b].T (host-relayouted [4,128,1024]) and
    accumulate logits[b, :] = qtld[:, b].T @ Zt in PSUM with qtld as the
    128x32 stationary operand (float32r -> full-rate streaming), then
    tanh on ScalarE and a final x10 on VectorE.
"""

import numpy as np
from contextlib import ExitStack

B, V, N, D = 256, 16, 1024, 512
NCORES = 8
CLIP = 10.0
P = 128
DC = D // P          # 4 chunks of 128 along D
KC = (3 * D) // P    # 12 chunks along 3D
NH = N // 512        # moving-operand halves of the node dim

_CACHE = {}


def _build(BS):
    """Build + compile the per-core Bass program for BS batches/core."""
    import concourse.bacc as bacc
    import concourse.tile as tile
    import concourse.mybir as mybir

    f32 = mybir.dt.float32
    f32r = mybir.dt.float32r
    Relu = mybir.ActivationFunctionType.Relu
    Tanh = mybir.ActivationFunctionType.Tanh

    nc = bacc.Bacc("TRN2", target_bir_lowering=False, debug=False,
                   num_devices=NCORES)

    zt = nc.dram_tensor("zt", [BS, DC, P, N], f32r, kind="ExternalInput").ap()
    gn = nc.dram_tensor("gn", [BS, D], f32, kind="ExternalInput").ap()
    gg = nc.dram_tensor("gg", [BS, D], f32, kind="ExternalInput").ap()
    zv = nc.dram_tensor("zv", [BS, V, D], f32, kind="ExternalInput").ap()
    w1t = nc.dram_tensor("w1t", [3 * D, D], f32, kind="ExternalInput").ap()
    wqt = nc.dram_tensor("wqt", [D, D], f32, kind="ExternalInput").ap()
    wk = nc.dram_tensor("wk", [D, D], f32, kind="ExternalInput").ap()
    bc = nc.dram_tensor("bc", [D], f32, kind="ExternalInput").ap()
    eye = nc.dram_tensor("eye", [32, 32], f32, kind="ExternalInput").ap()
    out = nc.dram_tensor("out", [BS, N], f32, kind="ExternalOutput").ap()

    with tile.TileContext(nc) as tc, ExitStack() as ctx:
        singles = ctx.enter_context(tc.tile_pool(name="singles", bufs=1))

        # ---- load replicated weights / per-core small inputs ----
        w1t_sb = singles.tile([P, KC, D], f32)
        nc.sync.dma_start(w1t_sb[:], w1t.rearrange("(kc p) j -> p kc j", p=P))
        wqt_sb = singles.tile([P, DC, D], f32)
        nc.sync.dma_start(wqt_sb[:], wqt.rearrange("(kc p) j -> p kc j", p=P))
        wk_sb = singles.tile([P, DC, D], f32)
        nc.sync.dma_start(wk_sb[:], wk.rearrange("(ec p) d -> p ec d", p=P))
        bc_sb = singles.tile([P, DC], f32)
        nc.sync.dma_start(bc_sb[:], bc.rearrange("(jc p) -> p jc", p=P))
        eye_sb = singles.tile([32, 32], f32)
        nc.sync.dma_start(eye_sb[:], eye[:])
        gn_sb = singles.tile([BS, D], f32)
        nc.sync.dma_start(gn_sb[:], gn[:])
        gg_sb = singles.tile([BS, D], f32)
        nc.sync.dma_start(gg_sb[:], gg[:])
        zv_sb = singles.tile([BS, V, D], f32)
        nc.sync.dma_start(zv_sb[:], zv[:])

        ident = eye_sb[:BS, :BS]
        pre_ps_cm = tc.tile_pool(name="pre_ps", bufs=2, space="PSUM")
        pre_ps = pre_ps_cm.__enter__()

        # ---- cat.T  [3D partition-chunks x BS] via PE transposes ----
        catT = singles.tile([P, KC, BS], f32)
        for dc in range(DC):
            ps = pre_ps.tile([P, BS], f32)
            nc.tensor.transpose(ps[:], gn_sb[:, dc * P:(dc + 1) * P], ident)
            nc.vector.tensor_copy(catT[:, dc, :], ps[:])
        for dc in range(DC):
            ps = pre_ps.tile([P, BS], f32)
            for v in range(V):
                nc.tensor.matmul(ps[:], zv_sb[:, v, dc * P:(dc + 1) * P],
                                 ident, start=(v == 0), stop=(v == V - 1),
                                 is_transpose=True)
            nc.vector.tensor_copy(catT[:, DC + dc, :], ps[:])
        for dc in range(DC):
            ps = pre_ps.tile([P, BS], f32)
            nc.tensor.transpose(ps[:], gg_sb[:, dc * P:(dc + 1) * P], ident)
            nc.vector.tensor_copy(catT[:, 2 * DC + dc, :], ps[:])

        # ---- ctx.T = relu(W_ctx @ cat.T + b_ctx) ----
        ctxT = singles.tile([P, DC, BS], f32)
        for jc in range(DC):
            ps = pre_ps.tile([P, BS], f32)
            for kc in range(KC):
                nc.tensor.matmul(ps[:], w1t_sb[:, kc, jc * P:(jc + 1) * P],
                                 catT[:, kc, :],
                                 start=(kc == 0), stop=(kc == KC - 1))
            nc.scalar.activation(ctxT[:, jc, :], ps[:], Relu,
                                 bias=bc_sb[:, jc:jc + 1], scale=1.0)

        # ---- Q.T = Wq @ ctx.T ----
        qT = singles.tile([P, DC, BS], f32)
        for jc in range(DC):
            ps = pre_ps.tile([P, BS], f32)
            for kc in range(DC):
                nc.tensor.matmul(ps[:], wqt_sb[:, kc, jc * P:(jc + 1) * P],
                                 ctxT[:, kc, :],
                                 start=(kc == 0), stop=(kc == DC - 1))
            nc.vector.tensor_copy(qT[:, jc, :], ps[:])

        # ---- qtld.T = (Wk/sqrt(D)).T @ Q.T  (scale folded host-side) ----
        qtld = singles.tile([P, DC, BS], f32r)
        for dc in range(DC):
            ps = pre_ps.tile([P, BS], f32)
            for ec in range(DC):
                nc.tensor.matmul(ps[:], wk_sb[:, ec, dc * P:(dc + 1) * P],
                                 qT[:, ec, :],
                                 start=(ec == 0), stop=(ec == DC - 1))
            nc.vector.tensor_copy(qtld[:, dc, :], ps[:])

        pre_ps_cm.__exit__(None, None, None)

        # ---- main loop: logits[b, :] = qtld[:, b] . Z_node[b].T ----
        # M=1 matmuls keep every engine access at partition 0 (engine SBUF
        # APs must start at partition 0/32/64/96).  Per-b tanh rows bounce
        # through a DRAM scratch tile to be re-gathered as [BS, N].
        zpool = ctx.enter_context(tc.tile_pool(name="z", bufs=4))
        lps = ctx.enter_context(tc.tile_pool(name="lps", bufs=4, space="PSUM"))
        tpool = ctx.enter_context(tc.tile_pool(name="tanh", bufs=3))
        dpool = ctx.enter_context(tc.tile_pool(name="dram", bufs=1,
                                               space="DRAM"))
        lg = dpool.tile([BS, N], f32)
        for b in range(BS):
            ztile = zpool.tile([P, DC, N], f32r)
            nc.sync.dma_start(ztile[:], zt[b].rearrange("dc p n -> p dc n"))
            ps = lps.tile([1, N], f32)
            for dc in range(DC):
                for nh in range(NH):
                    nc.tensor.matmul(
                        ps[:, nh * 512:(nh + 1) * 512],
                        qtld[:, dc, b:b + 1],
                        ztile[:, dc, nh * 512:(nh + 1) * 512],
                        start=(dc == 0), stop=(dc == DC - 1))
            trow = tpool.tile([1, N], f32)
            nc.scalar.activation(trow[:], ps[:], Tanh, scale=1.0)
            nc.sync.dma_start(lg[b:b + 1, :], trow[:])
        ld = singles.tile([BS, N], f32)
        nc.sync.dma_start(ld[:], lg[:])
        out_sb = singles.tile([BS, N], f32)
        nc.vector.tensor_scalar_mul(out_sb[:], ld[:], CLIP)
        nc.sync.dma_start(out[:], out_sb[:])

    nc.compile()
    return nc


def _get_nc(BS):
    if BS not in _CACHE:
        _CACHE[BS] = _build(BS)
    return _CACHE[BS]


def _make_in_maps(g_node, Z_veh, g_graph, Z_node, W_ctx, b_ctx, Wq, Wk, BS):
    ncores = g_node.shape[0] // BS
    w1t = np.ascontiguousarray(W_ctx.T)          # [3D, D], k-major
    w1t[D:2 * D, :] *= np.float32(1.0 / V)       # fold the Z_veh mean's 1/V
    wqt = np.ascontiguousarray(Wq.T)             # [D, D], contraction-major
    wk = np.ascontiguousarray(Wk * np.float32(1.0 / np.sqrt(D)))
    bc = np.ascontiguousarray(b_ctx)
    eye = np.eye(32, dtype=np.float32)

    in_maps = []
    for c in range(ncores):
        s = slice(c * BS, (c + 1) * BS)
        zt = np.ascontiguousarray(
            Z_node[s].transpose(0, 2, 1)).reshape(BS, DC, P, N)
        in_maps.append({
            "zt": zt,
            "gn": np.ascontiguousarray(g_node[s]),
            "gg": np.ascontiguousarray(g_graph[s]),
            "zv": np.ascontiguousarray(Z_veh[s]),
            "w1t": w1t, "wqt": wqt, "wk": wk, "bc": bc, "eye": eye,
        })
    return in_maps


def kernel(g_node, Z_veh, g_graph, Z_node, mask, W_ctx, b_ctx, Wq, Wk):
    from concourse.bass_utils import run_bass_kernel_spmd

    g_node = np.asarray(g_node, np.float32)
    Z_veh = np.asarray(Z_veh, np.float32)
    g_graph = np.asarray(g_graph, np.float32)
    Z_node = np.asarray(Z_node, np.float32)
    mask = np.asarray(mask, bool)
    W_ctx = np.asarray(W_ctx, np.float32)
    b_ctx = np.asarray(b_ctx, np.float32)
    Wq = np.asarray(Wq, np.float32)
    Wk = np.asarray(Wk, np.float32)

    BS = B // NCORES
    nc = _get_nc(BS)
    in_maps = _make_in_maps(g_node, Z_veh, g_graph, Z_node,
                            W_ctx, b_ctx, Wq, Wk, BS)
    res = run_bass_kernel_spmd(nc, in_maps, core_ids=list(range(NCORES)))
    logits = np.concatenate([r["out"] for r in res.results], axis=0)
    return np.where(mask, logits, np.float32(-np.inf)).astype(np.float32)
